# revision 13
# baseline (speedup 1.0000x reference)
"""Bistable recurrent cell layer on 8 Trainium2 NeuronCores.

Data-parallel over batch: each core owns B/8 = 8 batch rows, computes the
three input projections (x@kr, x@kz, x@kh) on the tensor engine, then runs
the T=512 sequential scan on DVE/ACT/GPSIMD, all in one NEFF.

Key tricks:
- Host pre-scales kz, bz, mz by 1/2 so z = sigmoid(xz + h*mz) becomes
  (tanh(sz')+1)/2 with sz' = xz' + h*mz' — every activation in the scan is
  a tanh, so the two first-stage activations fuse into one ACT instruction.
- The running state is stored as adjacent [h | h/2] column pairs so the
  fused wide-add reads both without broadcast APs.
- The scan is emitted as two independent batch-groups (b 0:4 / 4:8),
  interleaved, to hide the per-step cross-engine latency chain.
- Host pre-transposes x to [D, B_loc*T] per core (the GEMM contracts over
  d, which must live on partitions), and re-transposes outputs.
"""
import os
import sys

for _p in ('/opt/trn_rl_repo', os.path.dirname(os.path.abspath(__file__))):
    if _p not in sys.path:
        sys.path.insert(0, _p)

import numpy as np
from contextlib import ExitStack

import concourse.bass as bass
import concourse.tile as tile
from concourse import bacc, mybir
from concourse.bass_utils import run_bass_kernel_spmd

F32 = mybir.dt.float32
F32R = mybir.dt.float32r
AF = mybir.ActivationFunctionType
OP = mybir.AluOpType

B, T, D, H = 64, 512, 512, 512
NCORES = 8
BL = B // NCORES
NGRP = 2
BG = BL // NGRP

last_exec_time_ns = None


def _mm_cast(ap, use_f32r):
    return ap.bitcast(F32R) if use_f32r else ap


def build_body(ctx, tc, aps, cfg):
    nc = tc.nc
    Tt, TC, Bl = cfg['T'], cfg['TC'], cfg['BL']
    nchunk = Tt // TC
    use_f32r = cfg['use_f32r']
    use_gps = cfg.get('use_gps', True)

    weights = ctx.enter_context(tc.tile_pool(name='weights', bufs=1))
    xt_pool = ctx.enter_context(tc.tile_pool(name='xt', bufs=2))
    prod_pool = ctx.enter_context(tc.tile_pool(name='prod', bufs=2))
    ys_pool = ctx.enter_context(tc.tile_pool(name='ys', bufs=2))
    state = ctx.enter_context(tc.tile_pool(name='state', bufs=1))
    tmp = ctx.enter_context(tc.tile_pool(name='tmp', bufs=3))
    psum_pool = ctx.enter_context(tc.tile_pool(name='psum', bufs=6, space='PSUM'))

    dt_mm = F32R if use_f32r else F32

    # ---- weights: k order 0=r, 1=z(pre-halved), 2=h ----
    k_sb = []
    for name in ('kr', 'kz', 'kh'):
        t = weights.tile([128, 4, H], dt_mm, tag=name)
        nc.sync.dma_start(t[:], aps[name].rearrange('(dc p) h -> p dc h', p=128))
        k_sb.append(t)

    if cfg['general_bias']:
        b_sb = weights.tile([128, 2, 4], F32, tag='bias')  # [p, (r,z'), hb]
        nc.sync.dma_start(b_sb[:, 0, :], aps['br'].rearrange('(hb p) -> p hb', p=128))
        nc.sync.dma_start(b_sb[:, 1, :], aps['bz'].rearrange('(hb p) -> p hb', p=128))
    if cfg['general_m']:
        # [p, (mr, mz'), hb, b]
        m_sb = weights.tile([128, 2, 4, Bl], F32, tag='m')
        for i, nm in enumerate(('mr', 'mz')):
            src = aps[nm].rearrange('(hb p) -> p hb', p=128).unsqueeze(2)
            nc.sync.dma_start(m_sb[:, i, :, :], src.broadcast_to([128, 4, Bl]))

    # state pairs [h | h/2]: [p, pair, hb, b]
    h_last = state.tile([128, 2, 4, Bl], F32, tag='h_last')
    if cfg['general_h0']:
        h0_src = aps['h0'].rearrange('b (hb p) -> p hb b', p=128)
        for hb in range(4):
            nc.sync.dma_start(h_last[:, 0, hb], h0_src[:, hb])
        nc.vector.tensor_scalar_mul(h_last[:, 1], h_last[:, 0], 0.5)
    else:
        nc.vector.memset(h_last[:], 0.0)

    xt_src = aps['xt'].rearrange('(dc p) (b t) -> p dc b t', p=128, b=Bl)
    yt_dst = aps['yt'].rearrange('(hb p) (b t) -> p hb b t', p=128, b=Bl)

    for ci in range(nchunk):
        t0, t1_ = ci * TC, (ci + 1) * TC

        xt_t = xt_pool.tile([128, 4, Bl, TC], dt_mm, tag='xt')
        for dc in range(4):
            nc.sync.dma_start(xt_t[:, dc], xt_src[:, dc, :, t0:t1_])

        # prod [p, k(r,z',h), hb, b, t]
        prod = prod_pool.tile([128, 3, 4, Bl, TC], F32, tag='prod')
        icopy = 0
        for ht in range(4):
            for kj in range(3):
                ps = psum_pool.tile([128, Bl * TC], F32, tag='ps')
                for dc in range(4):
                    lhsT = k_sb[kj][:, dc, ht * 128:(ht + 1) * 128]
                    rhs = xt_t[:, dc, :, :]
                    nc.tensor.matmul(
                        ps[:], lhsT, rhs,
                        start=(dc == 0), stop=(dc == 3))
                dest = prod[:, kj, ht, :, :]
                ps_v = ps[:].rearrange('p (b t) -> p b t', b=Bl)
                if cfg['general_bias'] and kj < 2:
                    nc.scalar.activation(
                        dest, ps_v, AF.Identity, bias=b_sb[:, kj, ht:ht + 1])
                else:
                    if icopy % 3 == 0:
                        nc.scalar.copy(dest, ps_v)
                    else:
                        nc.vector.tensor_copy(dest, ps_v)
                    icopy += 1

        # ---- scan over this chunk, 2 interleaved batch-groups ----
        ys = ys_pool.tile([128, 2, 4, Bl, TC], F32, tag='ys')  # [p,pair,hb,b,t]
        for tt in range(TC):
            for g in range(NGRP):
                bs = slice(g * BG, (g + 1) * BG)
                hp = h_last[:, :, :, bs] if tt == 0 else ys[:, :, :, bs, tt - 1]
                h = hp[:, 0]
                AB = prod[:, 0:2, :, bs, tt]    # [128, 2, 4, BG]
                Ct = prod[:, 2, :, bs, tt]      # [128, 4, BG]

                szr = tmp.tile([128, 2, 4, BG], F32, tag=f'szr{g}')
                if cfg['general_m']:
                    hm = tmp.tile([128, 2, 4, BG], F32, tag=f'hm{g}')
                    nc.vector.tensor_mul(hm[:], hp, m_sb[:, :, :, bs])
                    nc.vector.tensor_add(szr[:], AB, hm[:])
                else:
                    nc.vector.tensor_add(szr[:], AB, hp)

                tz = tmp.tile([128, 2, 4, BG], F32, tag=f'tz{g}')
                nc.scalar.activation(tz[:], szr[:], AF.Tanh)
                t1 = tz[:, 0]
                tzz = tz[:, 1]

                rh = tmp.tile([128, 4, BG], F32, tag=f'rh{g}')
                nc.vector.scalar_tensor_tensor(
                    rh[:], t1, 1.0, h, OP.add, OP.mult)

                cc = tmp.tile([128, 4, BG], F32, tag=f'cc{g}')
                ee = tmp.tile([128, 4, BG], F32, tag=f'ee{g}')
                qq = tmp.tile([128, 4, BG], F32, tag=f'qq{g}')
                gg = tmp.tile([128, 4, BG], F32, tag=f'gg{g}')

                # TensorScalarPtr (STT/tensor_scalar) is not legal on Pool —
                # gpsimd only takes the plain tensor_tensor ops.
                eng = nc.gpsimd if use_gps else nc.vector
                eng.tensor_add(cc[:], rh[:], Ct)
                nc.scalar.activation(gg[:], cc[:], AF.Tanh)
                eng.tensor_sub(ee[:], h, gg[:])
                nc.vector.scalar_tensor_tensor(
                    qq[:], tzz, 1.0, ee[:], OP.add, OP.mult)
                # h' = g + q/2 ; hh' = h'/2
                nc.vector.scalar_tensor_tensor(
                    ys[:, 0, :, bs, tt], qq[:], 0.5, gg[:], OP.mult, OP.add)
                nc.vector.tensor_scalar_mul(
                    ys[:, 1, :, bs, tt], ys[:, 0, :, bs, tt], 0.5)

        nc.gpsimd.tensor_copy(h_last[:], ys[:, :, :, :, TC - 1])
        for hb in range(4):
            nc.sync.dma_start(yt_dst[:, hb, :, t0:t1_], ys[:, 0, hb])


def build_program(cfg):
    nc = bacc.Bacc('TRN2', target_bir_lowering=False, debug=False)
    Tt, Bl = cfg['T'], cfg['BL']
    aps = {}
    dt_mm = F32R if cfg['use_f32r'] else F32
    aps['xt'] = nc.dram_tensor('xt', [D, Bl * Tt], dt_mm,
                               kind='ExternalInput').ap()
    for name in ('kr', 'kz', 'kh'):
        aps[name] = nc.dram_tensor(name, [D, H], dt_mm,
                                   kind='ExternalInput').ap()
    if cfg['general_m']:
        for name in ('mr', 'mz'):
            aps[name] = nc.dram_tensor(name, [H], F32, kind='ExternalInput').ap()
    if cfg['general_bias']:
        for name in ('br', 'bz'):
            aps[name] = nc.dram_tensor(name, [H], F32, kind='ExternalInput').ap()
    if cfg['general_h0']:
        aps['h0'] = nc.dram_tensor('h0', [Bl, H], F32, kind='ExternalInput').ap()
    aps['yt'] = nc.dram_tensor('yt', [H, Bl * Tt], F32, kind='ExternalOutput').ap()

    with tile.TileContext(nc) as tc, ExitStack() as ctx:
        build_body(ctx, tc, aps, cfg)
    nc.compile()
    return nc


def _install_trace_hook():
    """Register the NTFF profile hook this image's antenv lacks, and neuter
    the cloud artifact upload, so trace=True works locally."""
    import types
    if 'antenv.axon_hooks' not in sys.modules:
        import antenv
        mod = types.ModuleType('antenv.axon_hooks')
        state = {'hook': None}
        mod.set_axon_ntff_profile_hook = lambda h: state.__setitem__('hook', h)
        mod.get_axon_ntff_profile_hook = lambda: state['hook']
        sys.modules['antenv.axon_hooks'] = mod
        antenv.axon_hooks = mod
        from trn_agent_boot.trn_boot import _ntff_profile_via_ctypes
        mod.set_axon_ntff_profile_hook(
            _ntff_profile_via_ctypes('/opt/axon/libaxon_pjrt.so'))
    import concourse.bass_utils as bu
    bu.upload_artifacts = lambda tmpdir: f"local:{tmpdir}"


_programs = {}


def _get_program(key, cfg):
    if key not in _programs:
        _programs[key] = build_program(cfg)
    return _programs[key]


def kernel(x, h0, kz, kr, kh, mz, mr, bz, br):
    global last_exec_time_ns
    x = np.asarray(x, dtype=np.float32)
    h0 = np.asarray(h0, dtype=np.float32)
    kz, kr, kh = (np.asarray(a, dtype=np.float32) for a in (kz, kr, kh))
    mz, mr, bz, br = (np.asarray(a, dtype=np.float32) for a in (mz, mr, bz, br))

    cfg = {
        'T': T, 'TC': int(os.environ.get('BRC_TC', '64')), 'BL': BL,
        'general_m': not (np.all(mz == 1.0) and np.all(mr == 1.0)),
        'general_bias': not (np.all(bz == 0.0) and np.all(br == 0.0)),
        'general_h0': not np.all(h0 == 0.0),
        'use_f32r': os.environ.get('BRC_F32R', '1') == '1',
        'use_gps': os.environ.get('BRC_NOGPS', '0') != '1',
    }
    key = tuple(sorted(cfg.items()))
    nc = _get_program(key, cfg)

    in_maps = []
    for c in range(NCORES):
        xi = x[c * BL:(c + 1) * BL]                      # [BL, T, D]
        xt = np.ascontiguousarray(
            xi.transpose(2, 0, 1).reshape(D, BL * T))     # [D, BL*T]
        m = {'xt': xt, 'kr': kr, 'kz': kz * 0.5, 'kh': kh}
        if cfg['general_m']:
            # the z-branch multiplies the h/2 pair column, so mz stays unscaled
            m['mr'] = mr
            m['mz'] = mz
        if cfg['general_bias']:
            m['br'] = br
            m['bz'] = bz * 0.5
        if cfg['general_h0']:
            m['h0'] = np.ascontiguousarray(h0[c * BL:(c + 1) * BL])
        in_maps.append(m)

    trace = os.environ.get('BRC_TRACE', '0') == '1'
    if trace:
        _install_trace_hook()
    res = run_bass_kernel_spmd(
        nc, in_maps, core_ids=list(range(NCORES)), trace=trace)
    last_exec_time_ns = res.exec_time_ns
    kernel.last_results = res

    out = np.empty((B, T, H), dtype=np.float32)
    for c in range(NCORES):
        yt = res.results[c]['yt']                         # [H, BL*T]
        out[c * BL:(c + 1) * BL] = (
            yt.reshape(H, BL, T).transpose(1, 2, 0))      # [BL, T, H]
    return out
